# revision 5
# baseline (speedup 1.0000x reference)
"""Trainium2 Bass kernel for a LISTA layer (nn_ListaLayer).

Reference computation (jax, fp32):
    th = relu(Theta) + 1e-7
    xW = (y @ W) / th
    repeat 16: z = xW + (unit_threshold(z) * th @ S) / th
    out = (unit_threshold(z) * th) @ Dx
where unit_threshold(v) = sign(v) * relu(|v| - 1).

Algebraic restructure (exact): track v = z * th.  Then
    v0 = y @ W
    repeat 16:  u = soft_threshold(v, th) = sign(v) * relu(|v| - th)
                v = v0 + u @ S
    out = soft_threshold(v, th) @ Dx
This removes every divide/multiply by th (soft_threshold(v,th) = relu(v-th) - relu(-v-th)).

Distribution: data-parallel over batch rows, 8 NeuronCores, 2048 rows each.
W/Theta/S/Dx replicated; no collectives.

On-chip layout is "transposed space": the dict dimension lives on SBUF
partitions and batch on the free axis, so the per-step matmul is
    vT = v0T + S^T-contract:  matmul(lhsT=S[jtile, itile], rhs=uT[jtile, :])
with S as the stationary operand and no per-step transposes.

Precision: matmuls run as split-fp16 (hi/lo) 3-pass accumulation in fp32 PSUM,
which is end-to-end indistinguishable from fp32 (~4e-6 max abs err; the fp32
reference itself wobbles ~1e-6 vs fp64).  PE fp16 subnormals are kept (measured
on HW), so no scaling of the low halves is needed.  The first K0_FP16 steps may
run as single-pass fp16 (early-step errors wash out through the contraction;
measured 3e-5 max abs at K0=8).  All splitting/transposition of inputs is done
host-side in numpy.
"""

import numpy as np
from contextlib import ExitStack

import concourse.bass as bass
import concourse.bacc as bacc
import concourse.tile as tile
import concourse.mybir as mybir
from concourse.bass import ts, ds

P = 128
NCORES = 8
B_FULL, DIN, DD = 16384, 1024, 2048
BSH = B_FULL // NCORES      # 2048 batch rows per core
CH = 256                    # batch columns per chunk (free dim of step matmuls)
NCH = BSH // CH             # 8 chunks
IT = DD // P                # 16 dict tiles
KW = DIN // P               # 8 d_in tiles
CN = 512                    # free dim of phase-C matmuls
K0_FP16 = 8                 # leading steps in single-pass fp16 (rest split-fp16)

F16 = mybir.dt.float16
F32 = mybir.dt.float32
ADD = mybir.AluOpType.add
SUB = mybir.AluOpType.subtract
RELU = mybir.ActivationFunctionType.Relu

_built = {}


def _build(steps: int):
    """Trace + schedule + compile the SPMD kernel for `steps` unroll steps."""
    nc = bacc.Bacc("TRN2", target_bir_lowering=False, debug=False, num_devices=NCORES)

    def inp(name, shape, dt):
        return nc.dram_tensor(name, shape, dt, kind="ExternalInput").ap()

    yTh = inp("yTh", (DIN, BSH), F16)
    yTl = inp("yTl", (DIN, BSH), F16)
    Wh_d = inp("Wh", (DIN, DD), F16)
    Wl_d = inp("Wl", (DIN, DD), F16)
    Sh_d = inp("Sh", (DD, DD), F16)
    Sl_d = inp("Sl", (DD, DD), F16)
    Dxh_d = inp("Dxh", (DD, DIN), F16)
    Dxl_d = inp("Dxl", (DD, DIN), F16)
    nth_d = inp("nth", (DD,), F32)   # -(relu(Theta) + eps)
    out_d = nc.dram_tensor("out", (BSH, DIN), F32, kind="ExternalOutput").ap()

    # step t (1-based) mode: 'h' = single-pass fp16, 's' = 3-pass split-fp16
    mode = ["h" if t <= K0_FP16 else "s" for t in range(1, steps + 1)]

    with tile.TileContext(nc) as tc, ExitStack() as top:
        dram = top.enter_context(tc.tile_pool(name="dram", bufs=1, space="DRAM"))
        v0_spill = dram.tile([IT, P, BSH], F32)
        ah_spill = dram.tile([IT, P, BSH], F16)
        al_spill = dram.tile([IT, P, BSH], F16)

        thp = top.enter_context(tc.tile_pool(name="thp", bufs=1))
        nth_t = thp.tile([P, IT], F32)
        nc.sync.dma_start(nth_t[:], nth_d.rearrange("(io p) -> p io", p=P))

        # ---------------- Phase A: v0T = W^T @ y^T -> DRAM spill ----------------
        with ExitStack() as ctx:
            wpool = ctx.enter_context(tc.tile_pool(name="wpool", bufs=1))
            ypool = ctx.enter_context(tc.tile_pool(name="ypool", bufs=2))
            psA = ctx.enter_context(tc.tile_pool(name="psA", bufs=2, space="PSUM"))
            stA = ctx.enter_context(tc.tile_pool(name="stA", bufs=3))

            Wh_t = wpool.tile([P, KW, DD], F16, name="Wh_t")
            Wl_t = wpool.tile([P, KW, DD], F16, name="Wl_t")
            for ko in range(KW):
                nc.sync.dma_start(Wh_t[:, ko, :], Wh_d[ts(ko, P), :])
                nc.sync.dma_start(Wl_t[:, ko, :], Wl_d[ts(ko, P), :])

            for c in range(NCH):
                cs = ds(c * CH, CH)
                yh_t = ypool.tile([P, KW, CH], F16, tag="yh")
                yl_t = ypool.tile([P, KW, CH], F16, tag="yl")
                for ko in range(KW):
                    nc.sync.dma_start(yh_t[:, ko, :], yTh[ts(ko, P), cs])
                    nc.sync.dma_start(yl_t[:, ko, :], yTl[ts(ko, P), cs])
                for i in range(IT):
                    ps = psA.tile([P, CH], F32, tag="psA")
                    n_mm = 3 * KW
                    k = 0
                    for ko in range(KW):
                        nc.tensor.matmul(ps[:], Wh_t[:, ko, ts(i, P)], yh_t[:, ko, :],
                                         start=(k == 0), stop=(k == n_mm - 1)); k += 1
                        nc.tensor.matmul(ps[:], Wh_t[:, ko, ts(i, P)], yl_t[:, ko, :],
                                         start=False, stop=(k == n_mm - 1)); k += 1
                    for ko in range(KW):
                        nc.tensor.matmul(ps[:], Wl_t[:, ko, ts(i, P)], yh_t[:, ko, :],
                                         start=False, stop=(k == n_mm - 1)); k += 1
                    st = stA.tile([P, CH], F32, tag="stA")
                    nc.vector.tensor_copy(st[:], ps[:])
                    nc.sync.dma_start(v0_spill[i, :, cs], st[:])

        # ---------------- Phase B: 16 soft-threshold + u@S steps ----------------
        with ExitStack() as ctx:
            spool = ctx.enter_context(tc.tile_pool(name="spool", bufs=1))
            v0pool = ctx.enter_context(tc.tile_pool(name="v0pool", bufs=2))
            upool = ctx.enter_context(tc.tile_pool(name="upool", bufs=2))
            psB = ctx.enter_context(tc.tile_pool(name="psB", bufs=2, space="PSUM"))
            vp = ctx.enter_context(tc.tile_pool(name="vp", bufs=3))
            pp = ctx.enter_context(tc.tile_pool(name="pp", bufs=3))
            qp = ctx.enter_context(tc.tile_pool(name="qp", bufs=3))
            u32p = ctx.enter_context(tc.tile_pool(name="u32p", bufs=3))

            Sh_t = spool.tile([P, IT, DD], F16, name="Sh_t")
            Sl_t = spool.tile([P, IT, DD], F16, name="Sl_t")
            for jo in range(IT):
                nc.sync.dma_start(Sh_t[:, jo, :], Sh_d[ts(jo, P), :])
                nc.sync.dma_start(Sl_t[:, jo, :], Sl_d[ts(jo, P), :])

            def shrink(v_ap, i, uh_n, ul_n):
                """u = relu(v - th) - relu(-v - th); write fp16 hi (and lo if ul_n)."""
                bias = nth_t[:, i:i + 1]
                p_t = pp.tile([P, CH], F32, tag="p")
                q_t = qp.tile([P, CH], F32, tag="q")
                nc.scalar.activation(p_t[:], v_ap, RELU, bias=bias)
                nc.scalar.activation(q_t[:], v_ap, RELU, bias=bias, scale=-1.0)
                if ul_n is None:
                    nc.vector.tensor_tensor(uh_n[:, i, :], p_t[:], q_t[:], SUB)
                else:
                    u32 = u32p.tile([P, CH], F32, tag="u32")
                    nc.vector.tensor_tensor(u32[:], p_t[:], q_t[:], SUB)
                    nc.vector.tensor_copy(uh_n[:, i, :], u32[:])
                    nc.vector.tensor_tensor(ul_n[:, i, :], u32[:], uh_n[:, i, :], SUB)

            for c in range(NCH):
                cs = ds(c * CH, CH)
                v0_t = v0pool.tile([P, IT, CH], F32, tag="v0")
                nc.sync.dma_start(v0_t[:], v0_spill[:, :, cs].rearrange("io p b -> p io b"))

                # u_1 = shrink(v0)
                need_l = mode[0] == "s"
                uh_c = upool.tile([P, IT, CH], F16, tag="uh", name="uh_c")
                ul_c = upool.tile([P, IT, CH], F16, tag="ul", name="ul_c") if need_l else None
                for i in range(IT):
                    shrink(v0_t[:, i, :], i, uh_c, ul_c)

                for t in range(steps):
                    m = mode[t]
                    # u consumed this step: uh_c (+ ul_c if split)
                    nxt_need_l = True if t == steps - 1 else (mode[t + 1] == "s")
                    uh_n = upool.tile([P, IT, CH], F16, tag="uh", name="uh_n")
                    ul_n = upool.tile([P, IT, CH], F16, tag="ul", name="ul_n") if nxt_need_l else None
                    for i in range(IT):
                        ps = psB.tile([P, CH], F32, tag="psB")
                        n_mm = IT * (3 if m == "s" else 1)
                        k = 0
                        if m == "s":
                            for jo in range(IT):
                                nc.tensor.matmul(ps[:], Sh_t[:, jo, ts(i, P)], uh_c[:, jo, :],
                                                 start=(k == 0), stop=(k == n_mm - 1)); k += 1
                                nc.tensor.matmul(ps[:], Sh_t[:, jo, ts(i, P)], ul_c[:, jo, :],
                                                 start=False, stop=(k == n_mm - 1)); k += 1
                            for jo in range(IT):
                                nc.tensor.matmul(ps[:], Sl_t[:, jo, ts(i, P)], uh_c[:, jo, :],
                                                 start=False, stop=(k == n_mm - 1)); k += 1
                        else:
                            for jo in range(IT):
                                nc.tensor.matmul(ps[:], Sh_t[:, jo, ts(i, P)], uh_c[:, jo, :],
                                                 start=(k == 0), stop=(k == n_mm - 1)); k += 1
                        v_t = vp.tile([P, CH], F32, tag="v")
                        nc.vector.tensor_tensor(v_t[:], ps[:], v0_t[:, i, :], ADD)
                        shrink(v_t[:], i, uh_n, ul_n)
                    uh_c, ul_c = uh_n, ul_n

                # after `steps` steps, (uh_c, ul_c) hold a = shrink(v_final)
                nc.sync.dma_start(ah_spill[:, :, cs].rearrange("io p b -> p io b"), uh_c[:])
                nc.sync.dma_start(al_spill[:, :, cs].rearrange("io p b -> p io b"), ul_c[:])

        # ---------------- Phase C: out = a @ Dx (normal orientation) ----------------
        with ExitStack() as ctx:
            dxpool = ctx.enter_context(tc.tile_pool(name="dxpool", bufs=1))
            apool = ctx.enter_context(tc.tile_pool(name="apool", bufs=2))
            psC = ctx.enter_context(tc.tile_pool(name="psC", bufs=2, space="PSUM"))
            stC = ctx.enter_context(tc.tile_pool(name="stC", bufs=3))

            Dxh_t = dxpool.tile([P, IT, DIN], F16, name="Dxh_t")
            Dxl_t = dxpool.tile([P, IT, DIN], F16, name="Dxl_t")
            for io in range(IT):
                nc.sync.dma_start(Dxh_t[:, io, :], Dxh_d[ts(io, P), :])
                nc.sync.dma_start(Dxl_t[:, io, :], Dxl_d[ts(io, P), :])

            for c in range(NCH):
                cs = ds(c * CH, CH)
                ah_c = apool.tile([P, IT, CH], F16, tag="ah")
                al_c = apool.tile([P, IT, CH], F16, tag="al")
                nc.sync.dma_start(ah_c[:], ah_spill[:, :, cs].rearrange("io p b -> p io b"))
                nc.sync.dma_start(al_c[:], al_spill[:, :, cs].rearrange("io p b -> p io b"))
                for bt in range(CH // P):
                    for dn in range(DIN // CN):
                        ps = psC.tile([P, CN], F32, tag="psC")
                        n_mm = 3 * IT
                        k = 0
                        for io in range(IT):
                            nc.tensor.matmul(ps[:], ah_c[:, io, ts(bt, P)],
                                             Dxh_t[:, io, ts(dn, CN)],
                                             start=(k == 0), stop=(k == n_mm - 1)); k += 1
                            nc.tensor.matmul(ps[:], ah_c[:, io, ts(bt, P)],
                                             Dxl_t[:, io, ts(dn, CN)],
                                             start=False, stop=(k == n_mm - 1)); k += 1
                        for io in range(IT):
                            nc.tensor.matmul(ps[:], al_c[:, io, ts(bt, P)],
                                             Dxh_t[:, io, ts(dn, CN)],
                                             start=False, stop=(k == n_mm - 1)); k += 1
                        st = stC.tile([P, CN], F32, tag="stC")
                        nc.vector.tensor_copy(st[:], ps[:])
                        nc.sync.dma_start(out_d[ds(c * CH + bt * P, P), ts(dn, CN)], st[:])

    nc.compile()
    return nc


def _split16(x):
    hi = x.astype(np.float16)
    lo = (x - hi.astype(np.float32)).astype(np.float16)
    return hi, lo


def _prep_in_maps(y, W, Theta, S, Dx):
    y = np.ascontiguousarray(np.asarray(y, dtype=np.float32))
    W = np.asarray(W, dtype=np.float32)
    Theta = np.asarray(Theta, dtype=np.float32)
    S = np.asarray(S, dtype=np.float32)
    Dx = np.asarray(Dx, dtype=np.float32)
    assert y.shape == (B_FULL, DIN) and W.shape == (DIN, DD)
    assert S.shape == (DD, DD) and Dx.shape == (DD, DIN)

    Wh, Wl = _split16(W)
    Sh, Sl = _split16(S)
    Dxh, Dxl = _split16(Dx)
    nth = -(np.maximum(Theta, 0.0) + np.float32(1e-7))
    yT = np.ascontiguousarray(y.T)          # [DIN, B]
    yTh_f, yTl_f = _split16(yT)

    shared = dict(Wh=Wh, Wl=Wl, Sh=Sh, Sl=Sl, Dxh=Dxh, Dxl=Dxl, nth=nth)
    in_maps = []
    for c in range(NCORES):
        sl = slice(c * BSH, (c + 1) * BSH)
        in_maps.append(dict(shared, yTh=np.ascontiguousarray(yTh_f[:, sl]),
                            yTl=np.ascontiguousarray(yTl_f[:, sl])))
    return in_maps


_sharded_cache = {}


def _get_sharded(steps: int):
    """Build (once) the jitted shard_map executable for the compiled NEFF.

    Mirrors concourse.bass2jax.run_bass_via_pjrt's multi-core path, but caches
    the jit so repeated kernel() calls don't re-trace/re-compile."""
    if steps in _sharded_cache:
        return _sharded_cache[steps]
    import jax
    from jax.experimental.shard_map import shard_map
    from jax.sharding import Mesh, PartitionSpec
    from concourse import bass2jax

    if steps not in _built:
        _built[steps] = _build(steps)
    nc = _built[steps]
    bass2jax.install_neuronx_cc_hook()
    assert nc.dbg_addr is None
    partition_name = nc.partition_id_tensor.name if nc.partition_id_tensor else None

    in_names, out_names, out_avals, zero_shapes = [], [], [], []
    for alloc in nc.m.functions[0].allocations:
        if not isinstance(alloc, mybir.MemoryLocationSet):
            continue
        name = alloc.memorylocations[0].name
        if alloc.kind == "ExternalInput":
            if name != partition_name:
                in_names.append(name)
        elif alloc.kind == "ExternalOutput":
            out_names.append(name)
            shape = tuple(alloc.tensor_shape)
            dtype = mybir.dt.np(alloc.dtype)
            out_avals.append(jax.core.ShapedArray(shape, dtype))
            zero_shapes.append((shape, dtype))
    n_params = len(in_names)
    n_outs = len(out_names)
    all_in_names = in_names + out_names
    if partition_name is not None:
        all_in_names.append(partition_name)

    def _body(*args):
        operands = list(args)
        if partition_name is not None:
            operands.append(bass2jax.partition_id_tensor())
        outs = bass2jax._bass_exec_p.bind(
            *operands,
            out_avals=tuple(out_avals),
            in_names=tuple(all_in_names),
            out_names=tuple(out_names),
            lowering_input_output_aliases=(),
            sim_require_finite=True,
            sim_require_nnan=True,
            nc=nc,
        )
        return tuple(outs)

    devices = jax.devices()[:NCORES]
    mesh = Mesh(np.asarray(devices), ("core",))
    donate = tuple(range(n_params, n_params + n_outs))
    sharded = jax.jit(
        shard_map(_body, mesh=mesh,
                  in_specs=(PartitionSpec("core"),) * (n_params + n_outs),
                  out_specs=(PartitionSpec("core"),) * n_outs,
                  check_rep=False),
        donate_argnums=donate, keep_unused=True)
    entry = dict(sharded=sharded, in_names=in_names, out_names=out_names,
                 zero_shapes=zero_shapes, mesh=mesh, n_params=n_params)
    _sharded_cache[steps] = entry
    return entry


def _concat_inputs(entry, in_maps):
    return [np.concatenate([np.asarray(in_maps[c][n]) for c in range(NCORES)], axis=0)
            for n in entry["in_names"]]


def _run(entry, concat_in):
    import jax.numpy as jnp
    zeros = [np.zeros((NCORES * s[0], *s[1:]), d) for s, d in entry["zero_shapes"]]
    out_arrs = entry["sharded"](*concat_in, *zeros)
    return out_arrs


def kernel(y, W, Theta, S, Dx, unroll_steps):
    steps = int(unroll_steps)
    entry = _get_sharded(steps)
    in_maps = _prep_in_maps(y, W, Theta, S, Dx)
    out_arrs = _run(entry, _concat_inputs(entry, in_maps))
    idx = entry["out_names"].index("out")
    return np.ascontiguousarray(np.asarray(out_arrs[idx]))  # [NCORES*BSH, DIN]


def time_kernel(np_inputs, iters=6):
    """Steady-state wall time per NEFF execution (ns), device-resident inputs."""
    import jax
    from jax.sharding import NamedSharding, PartitionSpec
    steps = int(np_inputs["unroll_steps"])
    entry = _get_sharded(steps)
    in_maps = _prep_in_maps(np_inputs["y"], np_inputs["W"], np_inputs["Theta"],
                            np_inputs["S"], np_inputs["Dx"])
    concat_in = _concat_inputs(entry, in_maps)
    sh = NamedSharding(entry["mesh"], PartitionSpec("core"))
    dev_in = [jax.device_put(a, sh) for a in concat_in]
    import time as _time
    times = []
    for it in range(iters):
        zeros = [jax.device_put(np.zeros((NCORES * s[0], *s[1:]), d), sh)
                 for s, d in entry["zero_shapes"]]
        for z in zeros:
            z.block_until_ready()
        t0 = _time.perf_counter()
        outs = entry["sharded"](*dev_in, *zeros)
        for o in outs:
            o.block_until_ready()
        times.append(_time.perf_counter() - t0)
    best = min(times[1:]) if len(times) > 1 else times[0]
    print("  per-iter times (ms):", [f"{t*1e3:.1f}" for t in times])
    return best * 1e9


if __name__ == "__main__":
    rng = np.random.default_rng(0)
    inputs = dict(
        y=rng.standard_normal((B_FULL, DIN), dtype=np.float32),
        W=(rng.standard_normal((DIN, DD)) * 0.02).astype(np.float32),
        Theta=rng.random(DD, dtype=np.float32),
        S=(rng.standard_normal((DD, DD)) * 0.02).astype(np.float32),
        Dx=(rng.standard_normal((DD, DIN)) * 0.02).astype(np.float32),
        unroll_steps=16,
    )
    out = kernel(**inputs)
    print("out", out.shape, out.dtype, np.abs(out).max())


# revision 6
# speedup vs baseline: 8.5093x; 8.5093x over previous
"""Trainium2 Bass kernel for a LISTA layer (nn_ListaLayer).

Reference computation (jax, fp32):
    th = relu(Theta) + 1e-7
    xW = (y @ W) / th
    repeat 16: z = xW + (unit_threshold(z) * th @ S) / th
    out = (unit_threshold(z) * th) @ Dx
where unit_threshold(v) = sign(v) * relu(|v| - 1).

Algebraic restructure (exact): track v = z * th.  Then
    v0 = y @ W
    repeat 16:  u = soft_threshold(v, th) = sign(v) * relu(|v| - th)
                v = v0 + u @ S
    out = soft_threshold(v, th) @ Dx
This removes every divide/multiply by th (soft_threshold(v,th) = relu(v-th) - relu(-v-th)).

Distribution: data-parallel over batch rows, 8 NeuronCores, 2048 rows each.
W/Theta/S/Dx replicated; no collectives.

On-chip layout is "transposed space": the dict dimension lives on SBUF
partitions and batch on the free axis, so the per-step matmul is
    vT = v0T + S^T-contract:  matmul(lhsT=S[jtile, itile], rhs=uT[jtile, :])
with S as the stationary operand and no per-step transposes.

Precision: matmuls run as split-fp16 (hi/lo) 3-pass accumulation in fp32 PSUM,
which is end-to-end indistinguishable from fp32 (~4e-6 max abs err; the fp32
reference itself wobbles ~1e-6 vs fp64).  PE fp16 subnormals are kept (measured
on HW), so no scaling of the low halves is needed.  The first K0_FP16 steps may
run as single-pass fp16 (early-step errors wash out through the contraction;
measured 3e-5 max abs at K0=8).  All splitting/transposition of inputs is done
host-side in numpy.
"""

import numpy as np
from contextlib import ExitStack

import concourse.bass as bass
import concourse.bacc as bacc
import concourse.tile as tile
import concourse.mybir as mybir
from concourse.bass import ts, ds

P = 128
NCORES = 8
B_FULL, DIN, DD = 16384, 1024, 2048
BSH = B_FULL // NCORES      # 2048 batch rows per core
CH = 256                    # batch columns per chunk (free dim of step matmuls)
NCH = BSH // CH             # 8 chunks
IT = DD // P                # 16 dict tiles
KW = DIN // P               # 8 d_in tiles
CN = 512                    # free dim of phase-C matmuls
K0_FP16 = 8                 # leading steps in single-pass fp16 (rest split-fp16)

F16 = mybir.dt.float16
F32 = mybir.dt.float32
ADD = mybir.AluOpType.add
SUB = mybir.AluOpType.subtract
RELU = mybir.ActivationFunctionType.Relu

_built = {}


def _build(steps: int):
    """Trace + schedule + compile the SPMD kernel for `steps` unroll steps."""
    nc = bacc.Bacc("TRN2", target_bir_lowering=False, debug=False, num_devices=NCORES)

    def inp(name, shape, dt):
        return nc.dram_tensor(name, shape, dt, kind="ExternalInput").ap()

    yTh = inp("yTh", (DIN, BSH), F16)
    yTl = inp("yTl", (DIN, BSH), F16)
    Wh_d = inp("Wh", (DIN, DD), F16)
    Wl_d = inp("Wl", (DIN, DD), F16)
    Sh_d = inp("Sh", (DD, DD), F16)
    Sl_d = inp("Sl", (DD, DD), F16)
    Dxh_d = inp("Dxh", (DD, DIN), F16)
    Dxl_d = inp("Dxl", (DD, DIN), F16)
    nth_d = inp("nth", (DD,), F32)   # -(relu(Theta) + eps)
    out_d = nc.dram_tensor("out", (BSH, DIN), F32, kind="ExternalOutput").ap()

    # step t (1-based) mode: 'h' = single-pass fp16, 's' = 3-pass split-fp16
    mode = ["h" if t <= K0_FP16 else "s" for t in range(1, steps + 1)]

    with tile.TileContext(nc) as tc, ExitStack() as top:
        dram = top.enter_context(tc.tile_pool(name="dram", bufs=1, space="DRAM"))
        v0_spill = dram.tile([IT, P, BSH], F32)
        ah_spill = dram.tile([IT, P, BSH], F16)
        al_spill = dram.tile([IT, P, BSH], F16)

        thp = top.enter_context(tc.tile_pool(name="thp", bufs=1))
        nth_t = thp.tile([P, IT], F32)
        nc.sync.dma_start(nth_t[:], nth_d.rearrange("(io p) -> p io", p=P))

        # ---------------- Phase A: v0T = W^T @ y^T -> DRAM spill ----------------
        with ExitStack() as ctx:
            wpool = ctx.enter_context(tc.tile_pool(name="wpool", bufs=1))
            ypool = ctx.enter_context(tc.tile_pool(name="ypool", bufs=2))
            psA = ctx.enter_context(tc.tile_pool(name="psA", bufs=4, space="PSUM"))
            stA = ctx.enter_context(tc.tile_pool(name="stA", bufs=3))

            Wh_t = wpool.tile([P, KW, DD], F16, name="Wh_t")
            Wl_t = wpool.tile([P, KW, DD], F16, name="Wl_t")
            for ko in range(KW):
                nc.sync.dma_start(Wh_t[:, ko, :], Wh_d[ts(ko, P), :])
                nc.sync.dma_start(Wl_t[:, ko, :], Wl_d[ts(ko, P), :])

            for c in range(NCH):
                cs = ds(c * CH, CH)
                yh_t = ypool.tile([P, KW, CH], F16, tag="yh")
                yl_t = ypool.tile([P, KW, CH], F16, tag="yl")
                for ko in range(KW):
                    nc.sync.dma_start(yh_t[:, ko, :], yTh[ts(ko, P), cs])
                    nc.sync.dma_start(yl_t[:, ko, :], yTl[ts(ko, P), cs])
                for i in range(IT):
                    ps = psA.tile([P, CH], F32, tag="psA")
                    n_mm = 3 * KW
                    k = 0
                    for ko in range(KW):
                        nc.tensor.matmul(ps[:], Wh_t[:, ko, ts(i, P)], yh_t[:, ko, :],
                                         start=(k == 0), stop=(k == n_mm - 1)); k += 1
                        nc.tensor.matmul(ps[:], Wh_t[:, ko, ts(i, P)], yl_t[:, ko, :],
                                         start=False, stop=(k == n_mm - 1)); k += 1
                    for ko in range(KW):
                        nc.tensor.matmul(ps[:], Wl_t[:, ko, ts(i, P)], yh_t[:, ko, :],
                                         start=False, stop=(k == n_mm - 1)); k += 1
                    st = stA.tile([P, CH], F32, tag="stA")
                    nc.vector.tensor_copy(st[:], ps[:])
                    nc.sync.dma_start(v0_spill[i, :, cs], st[:])

        # ---------------- Phase B: 16 soft-threshold + u@S steps ----------------
        with ExitStack() as ctx:
            spool = ctx.enter_context(tc.tile_pool(name="spool", bufs=1))
            v0pool = ctx.enter_context(tc.tile_pool(name="v0pool", bufs=2))
            upool = ctx.enter_context(tc.tile_pool(name="upool", bufs=2))
            psB = ctx.enter_context(tc.tile_pool(name="psB", bufs=4, space="PSUM"))
            vp = ctx.enter_context(tc.tile_pool(name="vp", bufs=3))
            pp = ctx.enter_context(tc.tile_pool(name="pp", bufs=3))
            qp = ctx.enter_context(tc.tile_pool(name="qp", bufs=3))
            u32p = ctx.enter_context(tc.tile_pool(name="u32p", bufs=3))

            Sh_t = spool.tile([P, IT, DD], F16, name="Sh_t")
            Sl_t = spool.tile([P, IT, DD], F16, name="Sl_t")
            for jo in range(IT):
                nc.sync.dma_start(Sh_t[:, jo, :], Sh_d[ts(jo, P), :])
                nc.sync.dma_start(Sl_t[:, jo, :], Sl_d[ts(jo, P), :])

            def shrink(v_ap, i, uh_n, ul_n):
                """u = relu(v - th) - relu(-v - th); write fp16 hi (and lo if ul_n)."""
                bias = nth_t[:, i:i + 1]
                p_t = pp.tile([P, CH], F32, tag="p")
                q_t = qp.tile([P, CH], F32, tag="q")
                nc.scalar.activation(p_t[:], v_ap, RELU, bias=bias)
                nc.scalar.activation(q_t[:], v_ap, RELU, bias=bias, scale=-1.0)
                if ul_n is None:
                    nc.vector.tensor_tensor(uh_n[:, i, :], p_t[:], q_t[:], SUB)
                else:
                    u32 = u32p.tile([P, CH], F32, tag="u32")
                    nc.vector.tensor_tensor(u32[:], p_t[:], q_t[:], SUB)
                    nc.vector.tensor_copy(uh_n[:, i, :], u32[:])
                    nc.vector.tensor_tensor(ul_n[:, i, :], u32[:], uh_n[:, i, :], SUB)

            for c in range(NCH):
                cs = ds(c * CH, CH)
                v0_t = v0pool.tile([P, IT, CH], F32, tag="v0")
                nc.sync.dma_start(v0_t[:], v0_spill[:, :, cs].rearrange("io p b -> p io b"))

                # u_1 = shrink(v0)
                need_l = mode[0] == "s"
                uh_c = upool.tile([P, IT, CH], F16, tag="uh", name="uh_c")
                ul_c = upool.tile([P, IT, CH], F16, tag="ul", name="ul_c") if need_l else None
                for i in range(IT):
                    shrink(v0_t[:, i, :], i, uh_c, ul_c)

                for t in range(steps):
                    m = mode[t]
                    # u consumed this step: uh_c (+ ul_c if split)
                    nxt_need_l = True if t == steps - 1 else (mode[t + 1] == "s")
                    uh_n = upool.tile([P, IT, CH], F16, tag="uh", name="uh_n")
                    ul_n = upool.tile([P, IT, CH], F16, tag="ul", name="ul_n") if nxt_need_l else None
                    for i in range(IT):
                        ps = psB.tile([P, CH], F32, tag="psB")
                        n_mm = IT * (3 if m == "s" else 1)
                        k = 0
                        if m == "s":
                            for jo in range(IT):
                                nc.tensor.matmul(ps[:], Sh_t[:, jo, ts(i, P)], uh_c[:, jo, :],
                                                 start=(k == 0), stop=(k == n_mm - 1)); k += 1
                                nc.tensor.matmul(ps[:], Sh_t[:, jo, ts(i, P)], ul_c[:, jo, :],
                                                 start=False, stop=(k == n_mm - 1)); k += 1
                            for jo in range(IT):
                                nc.tensor.matmul(ps[:], Sl_t[:, jo, ts(i, P)], uh_c[:, jo, :],
                                                 start=False, stop=(k == n_mm - 1)); k += 1
                        else:
                            for jo in range(IT):
                                nc.tensor.matmul(ps[:], Sh_t[:, jo, ts(i, P)], uh_c[:, jo, :],
                                                 start=(k == 0), stop=(k == n_mm - 1)); k += 1
                        v_t = vp.tile([P, CH], F32, tag="v")
                        nc.vector.tensor_tensor(v_t[:], ps[:], v0_t[:, i, :], ADD)
                        shrink(v_t[:], i, uh_n, ul_n)
                    uh_c, ul_c = uh_n, ul_n

                # after `steps` steps, (uh_c, ul_c) hold a = shrink(v_final)
                nc.sync.dma_start(ah_spill[:, :, cs].rearrange("io p b -> p io b"), uh_c[:])
                nc.sync.dma_start(al_spill[:, :, cs].rearrange("io p b -> p io b"), ul_c[:])

        # ---------------- Phase C: out = a @ Dx (normal orientation) ----------------
        with ExitStack() as ctx:
            dxpool = ctx.enter_context(tc.tile_pool(name="dxpool", bufs=1))
            apool = ctx.enter_context(tc.tile_pool(name="apool", bufs=2))
            psC = ctx.enter_context(tc.tile_pool(name="psC", bufs=3, space="PSUM"))
            stC = ctx.enter_context(tc.tile_pool(name="stC", bufs=3))

            Dxh_t = dxpool.tile([P, IT, DIN], F16, name="Dxh_t")
            Dxl_t = dxpool.tile([P, IT, DIN], F16, name="Dxl_t")
            for io in range(IT):
                nc.sync.dma_start(Dxh_t[:, io, :], Dxh_d[ts(io, P), :])
                nc.sync.dma_start(Dxl_t[:, io, :], Dxl_d[ts(io, P), :])

            for c in range(NCH):
                cs = ds(c * CH, CH)
                ah_c = apool.tile([P, IT, CH], F16, tag="ah")
                al_c = apool.tile([P, IT, CH], F16, tag="al")
                nc.sync.dma_start(ah_c[:], ah_spill[:, :, cs].rearrange("io p b -> p io b"))
                nc.sync.dma_start(al_c[:], al_spill[:, :, cs].rearrange("io p b -> p io b"))
                for bt in range(CH // P):
                    for dn in range(DIN // CN):
                        ps = psC.tile([P, CN], F32, tag="psC")
                        n_mm = 3 * IT
                        k = 0
                        for io in range(IT):
                            nc.tensor.matmul(ps[:], ah_c[:, io, ts(bt, P)],
                                             Dxh_t[:, io, ts(dn, CN)],
                                             start=(k == 0), stop=(k == n_mm - 1)); k += 1
                            nc.tensor.matmul(ps[:], ah_c[:, io, ts(bt, P)],
                                             Dxl_t[:, io, ts(dn, CN)],
                                             start=False, stop=(k == n_mm - 1)); k += 1
                        for io in range(IT):
                            nc.tensor.matmul(ps[:], al_c[:, io, ts(bt, P)],
                                             Dxh_t[:, io, ts(dn, CN)],
                                             start=False, stop=(k == n_mm - 1)); k += 1
                        st = stC.tile([P, CN], F32, tag="stC")
                        nc.vector.tensor_copy(st[:], ps[:])
                        nc.sync.dma_start(out_d[ds(c * CH + bt * P, P), ts(dn, CN)], st[:])

    nc.compile()
    return nc


def _split16(x):
    hi = x.astype(np.float16)
    lo = (x - hi.astype(np.float32)).astype(np.float16)
    return hi, lo


def _prep_in_maps(y, W, Theta, S, Dx):
    y = np.ascontiguousarray(np.asarray(y, dtype=np.float32))
    W = np.asarray(W, dtype=np.float32)
    Theta = np.asarray(Theta, dtype=np.float32)
    S = np.asarray(S, dtype=np.float32)
    Dx = np.asarray(Dx, dtype=np.float32)
    assert y.shape == (B_FULL, DIN) and W.shape == (DIN, DD)
    assert S.shape == (DD, DD) and Dx.shape == (DD, DIN)

    Wh, Wl = _split16(W)
    Sh, Sl = _split16(S)
    Dxh, Dxl = _split16(Dx)
    nth = -(np.maximum(Theta, 0.0) + np.float32(1e-7))
    yT = np.ascontiguousarray(y.T)          # [DIN, B]
    yTh_f, yTl_f = _split16(yT)

    shared = dict(Wh=Wh, Wl=Wl, Sh=Sh, Sl=Sl, Dxh=Dxh, Dxl=Dxl, nth=nth)
    in_maps = []
    for c in range(NCORES):
        sl = slice(c * BSH, (c + 1) * BSH)
        in_maps.append(dict(shared, yTh=np.ascontiguousarray(yTh_f[:, sl]),
                            yTl=np.ascontiguousarray(yTl_f[:, sl])))
    return in_maps


_sharded_cache = {}


def _get_sharded(steps: int):
    """Build (once) the jitted shard_map executable for the compiled NEFF.

    Mirrors concourse.bass2jax.run_bass_via_pjrt's multi-core path, but caches
    the jit so repeated kernel() calls don't re-trace/re-compile."""
    if steps in _sharded_cache:
        return _sharded_cache[steps]
    import jax
    from jax.experimental.shard_map import shard_map
    from jax.sharding import Mesh, PartitionSpec
    from concourse import bass2jax

    if steps not in _built:
        _built[steps] = _build(steps)
    nc = _built[steps]
    bass2jax.install_neuronx_cc_hook()
    assert nc.dbg_addr is None
    partition_name = nc.partition_id_tensor.name if nc.partition_id_tensor else None

    in_names, out_names, out_avals, zero_shapes = [], [], [], []
    for alloc in nc.m.functions[0].allocations:
        if not isinstance(alloc, mybir.MemoryLocationSet):
            continue
        name = alloc.memorylocations[0].name
        if alloc.kind == "ExternalInput":
            if name != partition_name:
                in_names.append(name)
        elif alloc.kind == "ExternalOutput":
            out_names.append(name)
            shape = tuple(alloc.tensor_shape)
            dtype = mybir.dt.np(alloc.dtype)
            out_avals.append(jax.core.ShapedArray(shape, dtype))
            zero_shapes.append((shape, dtype))
    n_params = len(in_names)
    n_outs = len(out_names)
    all_in_names = in_names + out_names
    if partition_name is not None:
        all_in_names.append(partition_name)

    def _body(*args):
        operands = list(args)
        if partition_name is not None:
            operands.append(bass2jax.partition_id_tensor())
        outs = bass2jax._bass_exec_p.bind(
            *operands,
            out_avals=tuple(out_avals),
            in_names=tuple(all_in_names),
            out_names=tuple(out_names),
            lowering_input_output_aliases=(),
            sim_require_finite=True,
            sim_require_nnan=True,
            nc=nc,
        )
        return tuple(outs)

    devices = jax.devices()[:NCORES]
    mesh = Mesh(np.asarray(devices), ("core",))
    donate = tuple(range(n_params, n_params + n_outs))
    sharded = jax.jit(
        shard_map(_body, mesh=mesh,
                  in_specs=(PartitionSpec("core"),) * (n_params + n_outs),
                  out_specs=(PartitionSpec("core"),) * n_outs,
                  check_rep=False),
        donate_argnums=donate, keep_unused=True)
    entry = dict(sharded=sharded, in_names=in_names, out_names=out_names,
                 zero_shapes=zero_shapes, mesh=mesh, n_params=n_params)
    _sharded_cache[steps] = entry
    return entry


def _concat_inputs(entry, in_maps):
    return [np.concatenate([np.asarray(in_maps[c][n]) for c in range(NCORES)], axis=0)
            for n in entry["in_names"]]


def _run(entry, concat_in):
    import jax.numpy as jnp
    zeros = [np.zeros((NCORES * s[0], *s[1:]), d) for s, d in entry["zero_shapes"]]
    out_arrs = entry["sharded"](*concat_in, *zeros)
    return out_arrs


def kernel(y, W, Theta, S, Dx, unroll_steps):
    steps = int(unroll_steps)
    entry = _get_sharded(steps)
    in_maps = _prep_in_maps(y, W, Theta, S, Dx)
    out_arrs = _run(entry, _concat_inputs(entry, in_maps))
    idx = entry["out_names"].index("out")
    return np.ascontiguousarray(np.asarray(out_arrs[idx]))  # [NCORES*BSH, DIN]


def time_kernel(np_inputs, iters=6):
    """Steady-state wall time per NEFF execution (ns), device-resident inputs."""
    import jax
    from jax.sharding import NamedSharding, PartitionSpec
    steps = int(np_inputs["unroll_steps"])
    entry = _get_sharded(steps)
    in_maps = _prep_in_maps(np_inputs["y"], np_inputs["W"], np_inputs["Theta"],
                            np_inputs["S"], np_inputs["Dx"])
    concat_in = _concat_inputs(entry, in_maps)
    sh = NamedSharding(entry["mesh"], PartitionSpec("core"))
    dev_in = [jax.device_put(a, sh) for a in concat_in]
    import time as _time
    times = []
    for it in range(iters):
        zeros = [jax.device_put(np.zeros((NCORES * s[0], *s[1:]), d), sh)
                 for s, d in entry["zero_shapes"]]
        for z in zeros:
            z.block_until_ready()
        t0 = _time.perf_counter()
        outs = entry["sharded"](*dev_in, *zeros)
        for o in outs:
            o.block_until_ready()
        times.append(_time.perf_counter() - t0)
    best = min(times[1:]) if len(times) > 1 else times[0]
    print("  per-iter times (ms):", [f"{t*1e3:.1f}" for t in times])
    return best * 1e9


if __name__ == "__main__":
    rng = np.random.default_rng(0)
    inputs = dict(
        y=rng.standard_normal((B_FULL, DIN), dtype=np.float32),
        W=(rng.standard_normal((DIN, DD)) * 0.02).astype(np.float32),
        Theta=rng.random(DD, dtype=np.float32),
        S=(rng.standard_normal((DD, DD)) * 0.02).astype(np.float32),
        Dx=(rng.standard_normal((DD, DIN)) * 0.02).astype(np.float32),
        unroll_steps=16,
    )
    out = kernel(**inputs)
    print("out", out.shape, out.dtype, np.abs(out).max())
